# revision 49
# baseline (speedup 1.0000x reference)
"""CBAM block (channel + spatial attention) Trainium2 Bass kernel.

Problem: x [32, 56, 56, 256] f32; data-parallel over batch across 8 NeuronCores
(4 images per core).  Everything is hardcoded for these shapes.

Per-core dataflow: 4 images = 2 image-PAIRS.  A pair is 2*3136 = 6272 = 128*49
rows, stored as X[p, q, t, c]: partition p in [0,128), pair q in [0,2),
t in [0,49), flat row within pair n = 49*p + t (so image 0 of the pair lives on
partitions [0,64), image 1 on [64,128)).  Each partition line of a pair DMA is
49*256*4 = 50KB contiguous DRAM -> near-peak HBM streaming.

Per pair, fully interleaved so pair 0's MLP runs while pair 1's stats stream:
  Phase A: amax = chunked strided DVE max-reduces (start while the second
    input chunk is still in flight) -> [128,256];
    channel sums via 25 PE matmuls (2 t-slices each, one-hot-half lhsT)
    -> psum [2(i),2,256], DVE fold -> ssum.  PE transposes + DVE 64-half
    reduces put both stats on channel partitions; tiny MLP (W1/relu/W2,
    sigmoid via ACT-bias chaining, no DVE in the chain) -> caT_both ->
    one transpose -> two constant selector matmuls -> bca [128, 256].
  Phase B (in-place, t-quarters for pipelining): GPSIMD X *= bca (broadcast
    over t via stride-0 AP); DVE tensor_reduce (axis=X) per quarter ->
    maxc/sumc.
  Conv 7x7 per image: maps -> DRAM -> h-major [112(ch,h),62(w, zero-pad 3)]
    tiles (channels packed); 7 full-width accumulated PE matmuls against
    host-expanded h-Toeplitz lhsT (1/C mean scale folded in); sa written
    back contiguously.  Map chains alternate sync/scalar DMA queues by pair.
  Apply: X *= sa in place (saf broadcast over c via stride-0 AP) in
    t-quarters: pair 0 on GPSIMD (DVE still on pair-1 stats), pair 1 on
    DVE with the last quarter on GPSIMD; output DMA drains per quarter.
"""

import os

import numpy as np

import concourse.bass as bass
import concourse.bacc as bacc
import concourse.tile as tile
from concourse import mybir
from concourse.bass_utils import run_bass_kernel_spmd

F32 = mybir.dt.float32
BF16 = mybir.dt.bfloat16
AX = mybir.AxisListType
OP = mybir.AluOpType
ACT = mybir.ActivationFunctionType

P = 128          # partitions
NPAIR = 2        # image pairs per core
T = 49           # rows per partition per pair (6272 = 128*49)
TH = 25          # B-phase half split
TSPL = 30        # apply split: GPSIMD does t<TSPL, DVE the rest
HP = 64          # partitions per image within a pair
C = 256          # channels
HW = 3136        # 56*56
NCORES = 8

_CACHE: dict = {}


def _bcast_t(ap: bass.AP, nt: int) -> bass.AP:
    """[p, c] AP -> [p, nt, c] with stride-0 broadcast over the middle axis."""
    assert len(ap.ap) == 2
    return bass.AP(tensor=ap.tensor, offset=ap.offset,
                   ap=[ap.ap[0], [0, nt], ap.ap[1]])


def _bcast_c(ap: bass.AP, nc_: int) -> bass.AP:
    """[p, t] AP -> [p, t, nc] with stride-0 broadcast over the last axis."""
    assert len(ap.ap) == 2
    return bass.AP(tensor=ap.tensor, offset=ap.offset,
                   ap=[ap.ap[0], ap.ap[1], [0, nc_]])


def _build_nc() -> bass.Bass:
    nc = bacc.Bacc()

    x_d = nc.dram_tensor("x", [4, 56, 56, C], F32, kind="ExternalInput")
    w1_d = nc.dram_tensor("w1", [C, 16], F32, kind="ExternalInput")
    b1_d = nc.dram_tensor("b1", [16], F32, kind="ExternalInput")
    w2_d = nc.dram_tensor("w2", [16, C], F32, kind="ExternalInput")
    b2_d = nc.dram_tensor("b2", [C], F32, kind="ExternalInput")
    # host-expanded Toeplitz bands of conv_w: [112(ch,w_in), 7(dh), 56(w_out)]
    tmat_d = nc.dram_tensor("tmat", [112, 7, 56], F32, kind="ExternalInput")
    out_d = nc.dram_tensor("out", [4, 56, 56, C], F32, kind="ExternalOutput")

    ident_d = nc.inline_tensor(np.eye(128, dtype=np.float32), name="ident128")
    # one-hot halves: col 0 selects image 0 (p<64), col 1 image 1
    ohp_np = np.zeros((128, 2), dtype=np.float32)
    ohp_np[:HP, 0] = 1.0
    ohp_np[HP:, 1] = 1.0
    ohp_d = nc.inline_tensor(ohp_np, name="ohp")
    # sel_j[k=(i*2+jj), p] = 1 iff jj == j and p in image-i half; broadcasts
    # ca4 [4(i,j), 128(c')] rows onto the right partitions/columns
    sel_np = np.zeros((2, 4, 128), dtype=np.float32)
    for j in range(2):
        for i in range(2):
            sel_np[j, i * 2 + j, i * HP : (i + 1) * HP] = 1.0
    sel0_d = nc.inline_tensor(sel_np[0], name="sel0")
    sel1_d = nc.inline_tensor(sel_np[1], name="sel1")

    # flat row-major views; pair q covers rows [6272q, 6272(q+1))
    x_flat = x_d[:].rearrange("b h w c -> (b h w) c")
    out_flat = out_d[:].rearrange("b h w c -> (b h w) c")

    with tile.TileContext(nc) as tc:
        import contextlib

        with contextlib.ExitStack() as ctx:
            cpool = ctx.enter_context(tc.tile_pool(name="cpool", bufs=1))
            xpool = ctx.enter_context(tc.tile_pool(name="xpool", bufs=1))
            work = ctx.enter_context(tc.tile_pool(name="work", bufs=2))
            small = ctx.enter_context(tc.tile_pool(name="small", bufs=4))
            mappool = ctx.enter_context(tc.tile_pool(name="mappool", bufs=2))
            psA = ctx.enter_context(tc.tile_pool(name="psA", bufs=2, space="PSUM"))
            psB = ctx.enter_context(tc.tile_pool(name="psB", bufs=2, space="PSUM"))
            psM = ctx.enter_context(tc.tile_pool(name="psM", bufs=2, space="PSUM"))
            dpool = ctx.enter_context(tc.tile_pool(name="dpool", bufs=2, space="DRAM"))

            # ---------------- big SBUF state ----------------
            X = xpool.tile([P, NPAIR, T, C], F32)

            # ---------------- DMA in FIRST (3 chunks per pair) ----------------
            for q in range(NPAIR):
                src = x_flat[q * P * T : (q + 1) * P * T, :].rearrange(
                    "(p t) c -> p t c", p=128
                )
                nc.sync.dma_start(out=X[:, q, 0:14, :], in_=src[:, 0:14, :])
                nc.sync.dma_start(out=X[:, q, 14:28, :], in_=src[:, 14:28, :])
                nc.sync.dma_start(out=X[:, q, 28:T, :], in_=src[:, 28:T, :])

            # ---------------- constants & weights ----------------
            ident = cpool.tile([128, 128], F32)
            nc.scalar.dma_start(out=ident, in_=ident_d[:])
            ohp = cpool.tile([128, 2], F32)
            nc.scalar.dma_start(out=ohp, in_=ohp_d[:])
            sel0 = cpool.tile([4, 128], F32)
            nc.scalar.dma_start(out=sel0, in_=sel0_d[:])
            sel1 = cpool.tile([4, 128], F32)
            nc.scalar.dma_start(out=sel1, in_=sel1_d[:])

            w1_sb = cpool.tile([128, 2, 16], F32)
            nc.scalar.dma_start(out=w1_sb, in_=w1_d[:].rearrange("(j p) m -> p j m", p=128))
            w2_sb = cpool.tile([16, 2, 128], F32)
            nc.scalar.dma_start(out=w2_sb, in_=w2_d[:].rearrange("k (j m) -> k j m", j=2))
            b1_sb = cpool.tile([16, 1], F32)
            nc.scalar.dma_start(out=b1_sb, in_=b1_d[:].rearrange("(p o) -> p o", o=1))
            b2_sb = cpool.tile([128, 2], F32)
            nc.scalar.dma_start(out=b2_sb, in_=b2_d[:].rearrange("(j p) -> p j", p=128))
            b2x2 = cpool.tile([128, 2], F32)
            nc.scalar.activation(out=b2x2, in_=b2_sb, func=ACT.Copy, scale=2.0)

            # Toeplitz conv matrices, host-expanded; channels stacked on 112
            # partitions: row k = 56*ch + w_in
            t_sb2 = cpool.tile([112, 7, 56], F32)
            nc.scalar.dma_start(out=t_sb2, in_=tmat_d[:])

            # PE warm-up matmuls touching every constant lhsT source so that
            # later matmuls don't accumulate one sync-wait per constant tensor
            # (the LDW struct has very few wait slots).
            pwu = psM.tile([128, 4], F32, tag="mlp")
            nc.tensor.matmul(pwu[0:2, 0:2], lhsT=ohp, rhs=ohp, start=True, stop=True)
            nc.tensor.matmul(
                pwu[0:128, 0:2], lhsT=sel0, rhs=sel0[:, 0:2], start=True, stop=True
            )
            nc.tensor.matmul(
                pwu[0:128, 0:2], lhsT=sel1, rhs=sel1[:, 0:2], start=True, stop=True
            )
            nc.tensor.matmul(
                pwu[0:4, 0:4], lhsT=ident[:, 0:4], rhs=ident[:, 0:4],
                start=True, stop=True,
            )
            nc.tensor.matmul(
                pwu[0:4, 0:4], lhsT=t_sb2[:, 0, 0:4], rhs=t_sb2[:, 0, 0:4],
                start=True, stop=True,
            )
            nc.tensor.matmul(
                pwu[0:4, 0:4], lhsT=w1_sb[:, 0, 0:4], rhs=w1_sb[:, 0, 0:4],
                start=True, stop=True,
            )
            nc.tensor.matmul(
                pwu[0:4, 0:4], lhsT=w2_sb[:, 0, 0:4], rhs=w2_sb[:, 0, 0:4],
                start=True, stop=True,
            )

            # ---------------- phase A + MLP, interleaved per pair ----------
            bca_all = []
            for q in range(NPAIR):
                # max over hw: strided reduce per input chunk, then combine
                amaxA = work.tile([P, C], F32, tag="amaxA")
                nc.vector.tensor_reduce(
                    out=amaxA,
                    in_=X[:, q, 0:14, :].rearrange("p t c -> p c t"),
                    axis=AX.X, op=OP.max,
                )
                amaxB = work.tile([P, C], F32, tag="amaxB")
                nc.vector.tensor_reduce(
                    out=amaxB,
                    in_=X[:, q, 14:28, :].rearrange("p t c -> p c t"),
                    axis=AX.X, op=OP.max,
                )
                amaxAB = work.tile([P, C], F32, tag="amaxAB")
                nc.vector.tensor_max(out=amaxAB, in0=amaxA, in1=amaxB)
                amaxC = work.tile([P, C], F32, tag="amaxC")
                nc.vector.tensor_reduce(
                    out=amaxC,
                    in_=X[:, q, 28:T, :].rearrange("p t c -> p c t"),
                    axis=AX.X, op=OP.max,
                )
                amax = work.tile([P, C], F32, tag="amax")
                nc.vector.tensor_max(out=amax, in0=amaxAB, in1=amaxC)
                # sum over hw on PE; psum [2(img), 2, 256] over 2-wide t-chunks
                ps2 = psA.tile([2, 2, C], F32, tag="ps_sum")
                ps2f = ps2.rearrange("i a c -> i (a c)")
                for k in range(24):
                    nc.tensor.matmul(
                        ps2f,
                        lhsT=ohp,
                        rhs=X[:, q, 2 * k : 2 * k + 2, :].rearrange("p t c -> p (t c)"),
                        start=(k == 0),
                        stop=False,
                    )
                nc.tensor.matmul(
                    ps2[:, 0, :], lhsT=ohp, rhs=X[:, q, 48, :],
                    start=False, stop=True,
                )
                ps2sb = small.tile([2, 2, C], F32, tag="ps2sb")
                nc.scalar.copy(out=ps2sb, in_=ps2)

                # partition max per image half: transpose then 64-wide reduces
                pamx = psM.tile([128, 2, 128], F32, tag="mlp")
                for j in range(2):
                    nc.tensor.transpose(
                        pamx[:, j, :], amax[:, j * 128 : (j + 1) * 128], ident
                    )
                mxT = small.tile([128, 2, 2], F32, tag="mxT")
                nc.vector.tensor_reduce(
                    out=mxT,
                    in_=pamx.rearrange("p j (i h) -> p j i h", i=2),
                    axis=AX.X, op=OP.max,
                )
                # transpose both sum rows, folding the two t-parity halves
                # in psum: pavg[c128, j, i] = sum_a ps2sb[i, a, j-chunk]^T
                pavg = psM.tile([128, 2, 2], F32, tag="mlp")
                for j in range(2):
                    for a in range(2):
                        nc.tensor.matmul(
                            pavg[:, j, :],
                            lhsT=ps2sb[:, a, j * 128 : (j + 1) * 128],
                            rhs=ident[0:2, 0:2],
                            start=(a == 0), stop=(a == 1),
                        )
                # statsT per image [c(2x128), j, (avg, max)]
                statsTs = []
                for i in range(2):
                    statsT = small.tile([128, 2, 2], F32, tag=f"statsT{i}")
                    nc.scalar.activation(
                        out=statsT[:, :, 0:1], in_=pavg[:, :, i : i + 1],
                        func=ACT.Copy, scale=1.0 / HW,
                    )
                    nc.scalar.copy(out=statsT[:, :, 1:2], in_=mxT[:, :, i : i + 1])
                    statsTs.append(statsT)

                # ---------------- MLP per image ----------------
                caT_both = small.tile([128, 4], F32, tag="caT_both")
                for i in range(2):
                    statsT = statsTs[i]
                    ph = psM.tile([16, 2], F32, tag="mlp")
                    for j in range(2):
                        nc.tensor.matmul(
                            ph, lhsT=w1_sb[:, j, :], rhs=statsT[:, j, :],
                            start=(j == 0), stop=(j == 1),
                        )
                    h_sb = small.tile([16, 2], F32, tag="h_sb")
                    nc.scalar.activation(
                        out=h_sb, in_=ph, func=ACT.Relu, bias=b1_sb, scale=1.0
                    )
                    for j in range(2):
                        pc = psM.tile([128, 2], F32, tag="mlp")
                        nc.tensor.matmul(
                            pc, lhsT=w2_sb[:, j, :], rhs=h_sb, start=True, stop=True
                        )
                        # (avg-path + 2*b2) via ACT bias, then sigmoid with the
                        # partial sum as bias — no DVE in the chain
                        catmp = small.tile([128, 1], F32, tag="catmp")
                        nc.scalar.activation(
                            out=catmp, in_=pc[:, 0:1], func=ACT.Identity,
                            bias=b2x2[:, j : j + 1], scale=1.0,
                        )
                        nc.scalar.activation(
                            out=caT_both[:, 2 * i + j : 2 * i + j + 1],
                            in_=pc[:, 1:2], func=ACT.Sigmoid,
                            bias=catmp, scale=1.0,
                        )

                pca4 = psM.tile([4, 128], F32, tag="mlp")
                nc.tensor.transpose(pca4, caT_both, ident)
                ca4 = small.tile([4, 128], F32, tag="ca4")
                nc.scalar.copy(out=ca4, in_=pca4)
                pbca = psB.tile([P, 2, 128], F32, tag="pbca")
                nc.tensor.matmul(pbca[:, 0, :], lhsT=sel0, rhs=ca4, start=True, stop=True)
                nc.tensor.matmul(pbca[:, 1, :], lhsT=sel1, rhs=ca4, start=True, stop=True)
                bca = work.tile([P, C], F32, tag="bca")
                nc.scalar.copy(out=bca, in_=pbca.rearrange("p j m -> p (j m)"))
                bca_all.append(bca)

            # ---------------- phase B (in place, t-quarters) ----------------
            saf_all = []
            for q in range(NPAIR):
                bca = bca_all[q]
                maxc = mappool.tile([P, T], F32, tag="maxc")
                sumc = mappool.tile([P, T], F32, tag="sumc")
                for t0, t1 in ((0, 8), (8, 20), (20, 34), (34, T)):
                    nc.gpsimd.tensor_mul(
                        out=X[:, q, t0:t1, :],
                        in0=X[:, q, t0:t1, :],
                        in1=_bcast_t(bca[:, :], t1 - t0),
                    )
                    nc.vector.tensor_reduce(
                        out=maxc[:, t0:t1], in_=X[:, q, t0:t1, :],
                        axis=AX.X, op=OP.max,
                    )
                    nc.vector.tensor_reduce(
                        out=sumc[:, t0:t1], in_=X[:, q, t0:t1, :],
                        axis=AX.X, op=OP.add,
                    )

                # ---- rearrange maps: [128, 49] -> per-image [112(w,ch), 56(h)]
                mq = nc.sync if q == 0 else nc.scalar
                mdr = dpool.tile([2, P * T], F32, tag="mdr")
                mq.dma_start(
                    out=mdr[0, :].rearrange("(p t) -> p t", p=128), in_=sumc
                )
                mq.dma_start(
                    out=mdr[1, :].rearrange("(p t) -> p t", p=128), in_=maxc
                )
                sdr = dpool.tile([P * T], F32, tag="sdr")
                for i in range(2):
                    # conv input, h-major, w zero-padded by 3 on both sides
                    cinH = work.tile([112, 62], F32, tag="cinH")
                    nc.vector.memset(cinH[:, 0:3], 0.0)
                    nc.vector.memset(cinH[:, 59:62], 0.0)
                    for ch in range(2):
                        mq.dma_start(
                            out=cinH[56 * ch : 56 * (ch + 1), 3:59],
                            in_=mdr[ch, i * HW : (i + 1) * HW].rearrange(
                                "(h w) -> h w", h=56
                            ),
                        )
                    # ---- conv: 7 full-width accumulated matmuls ----
                    pconv = psB.tile([56, 56], F32, tag="pconv")
                    for dwi in range(7):
                        nc.tensor.matmul(
                            pconv,
                            lhsT=t_sb2[:, dwi, :],
                            rhs=cinH[:, dwi : dwi + 56],
                            start=(dwi == 0), stop=(dwi == 6),
                        )
                    sawh = work.tile([56, 56], F32, tag="sawh")
                    nc.scalar.activation(out=sawh, in_=pconv, func=ACT.Sigmoid)
                    mq.dma_start(
                        out=sdr[i * HW : (i + 1) * HW].rearrange("(h w) -> h w", h=56),
                        in_=sawh,
                    )
                saf = mappool.tile([P, T], F32, tag="saf")
                mq.dma_start(out=saf, in_=sdr.rearrange("(p t) -> p t", p=128))
                saf_all.append(saf)

            # ---------------- apply (in place) + DMA out ----------------
            for q in range(NPAIR):
                saf = saf_all[q]
                dst = out_flat[q * P * T : (q + 1) * P * T, :].rearrange(
                    "(p t) c -> p t c", p=128
                )
                # quarters: pair 0 on GPSIMD (DVE still on pair-1 stats);
                # pair 1 fans the tail across DVE / ACT-blocks / GPSIMD
                oq = nc.sync if q == 0 else nc.scalar
                engs = ((nc.gpsimd, nc.gpsimd, nc.vector, nc.vector) if q == 0
                        else (nc.gpsimd, nc.gpsimd, nc.vector, nc.vector))
                for (t0, t1), eng in zip(((0, 13), (13, TH), (TH, 37), (37, T)), engs):
                    if eng is None:
                        for t in range(t0, t1):
                            nc.scalar.activation(
                                out=X[:, q, t, :], in_=X[:, q, t, :],
                                func=ACT.Copy, scale=saf[:, t : t + 1],
                            )
                    else:
                        eng.tensor_mul(
                            out=X[:, q, t0:t1, :],
                            in0=X[:, q, t0:t1, :],
                            in1=_bcast_c(saf[:, t0:t1], C),
                        )
                    oq.dma_start(out=dst[:, t0:t1, :], in_=X[:, q, t0:t1, :])

    nc.finalize()
    return nc


LAST_RESULTS = None


def kernel(x, w1, b1, w2, b2, conv_w):
    global LAST_RESULTS
    nc = _CACHE.get("nc")
    if nc is None:
        nc = _build_nc()
        _CACHE["nc"] = nc

    x = np.ascontiguousarray(np.asarray(x, dtype=np.float32))
    shards = np.split(x, NCORES, axis=0)
    # host-expanded h-Toeplitz bands: tmat[56*ch + h_in, dwi, h_out] =
    #   conv_w[h_in - h_out + 3, dwi, ch] for |h_in - h_out| <= 3 else 0
    cw = np.asarray(conv_w, dtype=np.float32).reshape(7, 7, 2)
    tmat = np.zeros((2, 56, 7, 56), dtype=np.float32)
    for dhi in range(7):
        dh = dhi - 3
        for h_out in range(max(0, -dh), 56 - max(0, dh)):
            tmat[:, h_out + dh, :, h_out] = cw[dhi, :, :].T
    tmat[0] /= C  # fold the mean-map 1/C into the conv weights
    common = {
        "w1": np.ascontiguousarray(np.asarray(w1, dtype=np.float32)),
        "b1": np.ascontiguousarray(np.asarray(b1, dtype=np.float32)),
        "w2": np.ascontiguousarray(np.asarray(w2, dtype=np.float32)),
        "b2": np.ascontiguousarray(np.asarray(b2, dtype=np.float32)),
        "tmat": np.ascontiguousarray(tmat.reshape(112, 7, 56)),
    }
    in_maps = [dict(common, x=np.ascontiguousarray(s)) for s in shards]

    res = run_bass_kernel_spmd(
        nc,
        in_maps,
        core_ids=list(range(NCORES)),
        trace=bool(int(os.environ.get("CBAM_TRACE", "0"))),
    )
    LAST_RESULTS = res
    return np.concatenate([r["out"] for r in res.results], axis=0)


# revision 50
# speedup vs baseline: 1.0206x; 1.0206x over previous
"""CBAM block (channel + spatial attention) Trainium2 Bass kernel.

Problem: x [32, 56, 56, 256] f32; data-parallel over batch across 8 NeuronCores
(4 images per core).  Everything is hardcoded for these shapes.

Per-core dataflow: 4 images = 2 image-PAIRS.  A pair is 2*3136 = 6272 = 128*49
rows, stored as X[p, q, t, c]: partition p in [0,128), pair q in [0,2),
t in [0,49), flat row within pair n = 49*p + t (so image 0 of the pair lives on
partitions [0,64), image 1 on [64,128)).  Each partition line of a pair DMA is
49*256*4 = 50KB contiguous DRAM -> near-peak HBM streaming.

Per pair, fully interleaved so pair 0's MLP runs while pair 1's stats stream:
  Phase A: amax = chunked strided DVE max-reduces (start while the second
    input chunk is still in flight) -> [128,256];
    channel sums via 25 PE matmuls (2 t-slices each, one-hot-half lhsT)
    -> psum [2(i),2,256], DVE fold -> ssum.  PE transposes + DVE 64-half
    reduces put both stats on channel partitions; tiny MLP (W1/relu/W2,
    sigmoid via ACT-bias chaining, no DVE in the chain) -> caT_both ->
    one transpose -> two constant selector matmuls -> bca [128, 256].
  Phase B (in-place, t-quarters for pipelining): GPSIMD X *= bca (broadcast
    over t via stride-0 AP); DVE tensor_reduce (axis=X) per quarter ->
    maxc/sumc.
  Conv 7x7 per image: maps -> DRAM -> h-major [112(ch,h),62(w, zero-pad 3)]
    tiles (channels packed); 7 full-width accumulated PE matmuls against
    host-expanded h-Toeplitz lhsT (1/C mean scale folded in); sa written
    back contiguously.  Map chains alternate sync/scalar DMA queues by pair.
  Apply: X *= sa in place (saf broadcast over c via stride-0 AP) in
    t-quarters: pair 0 on GPSIMD (DVE still on pair-1 stats), pair 1 on
    DVE with the last quarter on GPSIMD; output DMA drains per quarter.
"""

import os

import numpy as np

import concourse.bass as bass
import concourse.bacc as bacc
import concourse.tile as tile
from concourse import mybir
from concourse.bass_utils import run_bass_kernel_spmd

F32 = mybir.dt.float32
BF16 = mybir.dt.bfloat16
AX = mybir.AxisListType
OP = mybir.AluOpType
ACT = mybir.ActivationFunctionType

P = 128          # partitions
NPAIR = 2        # image pairs per core
T = 49           # rows per partition per pair (6272 = 128*49)
TH = 25          # B-phase half split
TSPL = 30        # apply split: GPSIMD does t<TSPL, DVE the rest
HP = 64          # partitions per image within a pair
C = 256          # channels
HW = 3136        # 56*56
NCORES = 8

_CACHE: dict = {}


def _bcast_t(ap: bass.AP, nt: int) -> bass.AP:
    """[p, c] AP -> [p, nt, c] with stride-0 broadcast over the middle axis."""
    assert len(ap.ap) == 2
    return bass.AP(tensor=ap.tensor, offset=ap.offset,
                   ap=[ap.ap[0], [0, nt], ap.ap[1]])


def _bcast_c(ap: bass.AP, nc_: int) -> bass.AP:
    """[p, t] AP -> [p, t, nc] with stride-0 broadcast over the last axis."""
    assert len(ap.ap) == 2
    return bass.AP(tensor=ap.tensor, offset=ap.offset,
                   ap=[ap.ap[0], ap.ap[1], [0, nc_]])


def _build_nc() -> bass.Bass:
    nc = bacc.Bacc()

    x_d = nc.dram_tensor("x", [4, 56, 56, C], F32, kind="ExternalInput")
    w1_d = nc.dram_tensor("w1", [C, 16], F32, kind="ExternalInput")
    b1_d = nc.dram_tensor("b1", [16], F32, kind="ExternalInput")
    w2_d = nc.dram_tensor("w2", [16, C], F32, kind="ExternalInput")
    b2_d = nc.dram_tensor("b2", [C], F32, kind="ExternalInput")
    # host-expanded Toeplitz bands of conv_w: [112(ch,w_in), 7(dh), 56(w_out)]
    tmat_d = nc.dram_tensor("tmat", [112, 7, 56], F32, kind="ExternalInput")
    out_d = nc.dram_tensor("out", [4, 56, 56, C], F32, kind="ExternalOutput")

    ident_d = nc.inline_tensor(np.eye(128, dtype=np.float32), name="ident128")
    # one-hot halves: col 0 selects image 0 (p<64), col 1 image 1
    ohp_np = np.zeros((128, 2), dtype=np.float32)
    ohp_np[:HP, 0] = 1.0
    ohp_np[HP:, 1] = 1.0
    ohp_d = nc.inline_tensor(ohp_np, name="ohp")
    # sel_j[k=(i*2+jj), p] = 1 iff jj == j and p in image-i half; broadcasts
    # ca4 [4(i,j), 128(c')] rows onto the right partitions/columns
    sel_np = np.zeros((2, 4, 128), dtype=np.float32)
    for j in range(2):
        for i in range(2):
            sel_np[j, i * 2 + j, i * HP : (i + 1) * HP] = 1.0
    sel0_d = nc.inline_tensor(sel_np[0], name="sel0")
    sel1_d = nc.inline_tensor(sel_np[1], name="sel1")

    # flat row-major views; pair q covers rows [6272q, 6272(q+1))
    x_flat = x_d[:].rearrange("b h w c -> (b h w) c")
    out_flat = out_d[:].rearrange("b h w c -> (b h w) c")

    with tile.TileContext(nc) as tc:
        import contextlib

        with contextlib.ExitStack() as ctx:
            cpool = ctx.enter_context(tc.tile_pool(name="cpool", bufs=1))
            xpool = ctx.enter_context(tc.tile_pool(name="xpool", bufs=1))
            work = ctx.enter_context(tc.tile_pool(name="work", bufs=2))
            small = ctx.enter_context(tc.tile_pool(name="small", bufs=4))
            mappool = ctx.enter_context(tc.tile_pool(name="mappool", bufs=2))
            psA = ctx.enter_context(tc.tile_pool(name="psA", bufs=2, space="PSUM"))
            psB = ctx.enter_context(tc.tile_pool(name="psB", bufs=2, space="PSUM"))
            psM = ctx.enter_context(tc.tile_pool(name="psM", bufs=2, space="PSUM"))
            dpool = ctx.enter_context(tc.tile_pool(name="dpool", bufs=2, space="DRAM"))

            # ---------------- big SBUF state ----------------
            X = xpool.tile([P, NPAIR, T, C], F32)

            # ---------------- DMA in FIRST (3 chunks per pair) ----------------
            for q in range(NPAIR):
                src = x_flat[q * P * T : (q + 1) * P * T, :].rearrange(
                    "(p t) c -> p t c", p=128
                )
                nc.sync.dma_start(out=X[:, q, 0:14, :], in_=src[:, 0:14, :])
                nc.sync.dma_start(out=X[:, q, 14:28, :], in_=src[:, 14:28, :])
                nc.sync.dma_start(out=X[:, q, 28:T, :], in_=src[:, 28:T, :])

            # ---------------- constants & weights ----------------
            ident = cpool.tile([128, 128], F32)
            nc.scalar.dma_start(out=ident, in_=ident_d[:])
            ohp = cpool.tile([128, 2], F32)
            nc.scalar.dma_start(out=ohp, in_=ohp_d[:])
            sel0 = cpool.tile([4, 128], F32)
            nc.scalar.dma_start(out=sel0, in_=sel0_d[:])
            sel1 = cpool.tile([4, 128], F32)
            nc.scalar.dma_start(out=sel1, in_=sel1_d[:])

            w1_sb = cpool.tile([128, 2, 16], F32)
            nc.scalar.dma_start(out=w1_sb, in_=w1_d[:].rearrange("(j p) m -> p j m", p=128))
            w2_sb = cpool.tile([16, 2, 128], F32)
            nc.scalar.dma_start(out=w2_sb, in_=w2_d[:].rearrange("k (j m) -> k j m", j=2))
            b1_sb = cpool.tile([16, 1], F32)
            nc.scalar.dma_start(out=b1_sb, in_=b1_d[:].rearrange("(p o) -> p o", o=1))
            b2_sb = cpool.tile([128, 2], F32)
            nc.scalar.dma_start(out=b2_sb, in_=b2_d[:].rearrange("(j p) -> p j", p=128))
            b2x2 = cpool.tile([128, 2], F32)
            nc.scalar.activation(out=b2x2, in_=b2_sb, func=ACT.Copy, scale=2.0)

            # Toeplitz conv matrices, host-expanded; channels stacked on 112
            # partitions: row k = 56*ch + w_in
            t_sb2 = cpool.tile([112, 7, 56], F32)
            nc.scalar.dma_start(out=t_sb2, in_=tmat_d[:])

            # PE warm-up matmuls touching every constant lhsT source so that
            # later matmuls don't accumulate one sync-wait per constant tensor
            # (the LDW struct has very few wait slots).
            pwu = psM.tile([128, 4], F32, tag="mlp")
            nc.tensor.matmul(pwu[0:2, 0:2], lhsT=ohp, rhs=ohp, start=True, stop=True)
            nc.tensor.matmul(
                pwu[0:128, 0:2], lhsT=sel0, rhs=sel0[:, 0:2], start=True, stop=True
            )
            nc.tensor.matmul(
                pwu[0:128, 0:2], lhsT=sel1, rhs=sel1[:, 0:2], start=True, stop=True
            )
            nc.tensor.matmul(
                pwu[0:4, 0:4], lhsT=ident[:, 0:4], rhs=ident[:, 0:4],
                start=True, stop=True,
            )
            nc.tensor.matmul(
                pwu[0:4, 0:4], lhsT=t_sb2[:, 0, 0:4], rhs=t_sb2[:, 0, 0:4],
                start=True, stop=True,
            )
            nc.tensor.matmul(
                pwu[0:4, 0:4], lhsT=w1_sb[:, 0, 0:4], rhs=w1_sb[:, 0, 0:4],
                start=True, stop=True,
            )
            nc.tensor.matmul(
                pwu[0:4, 0:4], lhsT=w2_sb[:, 0, 0:4], rhs=w2_sb[:, 0, 0:4],
                start=True, stop=True,
            )

            # ---------------- phase A + MLP, interleaved per pair ----------
            bca_all = []
            for q in range(NPAIR):
                # max over hw: strided reduce per input chunk, then combine
                amaxA = work.tile([P, C], F32, tag="amaxA")
                nc.vector.tensor_reduce(
                    out=amaxA,
                    in_=X[:, q, 0:14, :].rearrange("p t c -> p c t"),
                    axis=AX.X, op=OP.max,
                )
                amaxB = work.tile([P, C], F32, tag="amaxB")
                nc.vector.tensor_reduce(
                    out=amaxB,
                    in_=X[:, q, 14:28, :].rearrange("p t c -> p c t"),
                    axis=AX.X, op=OP.max,
                )
                amaxAB = work.tile([P, C], F32, tag="amaxAB")
                nc.vector.tensor_max(out=amaxAB, in0=amaxA, in1=amaxB)
                amaxC = work.tile([P, C], F32, tag="amaxC")
                nc.vector.tensor_reduce(
                    out=amaxC,
                    in_=X[:, q, 28:T, :].rearrange("p t c -> p c t"),
                    axis=AX.X, op=OP.max,
                )
                amax = work.tile([P, C], F32, tag="amax")
                nc.vector.tensor_max(out=amax, in0=amaxAB, in1=amaxC)
                # sum over hw on PE; psum [2(img), 2, 256] over 2-wide t-chunks
                ps2 = psA.tile([2, 2, C], F32, tag="ps_sum")
                ps2f = ps2.rearrange("i a c -> i (a c)")
                for k in range(24):
                    nc.tensor.matmul(
                        ps2f,
                        lhsT=ohp,
                        rhs=X[:, q, 2 * k : 2 * k + 2, :].rearrange("p t c -> p (t c)"),
                        start=(k == 0),
                        stop=False,
                    )
                nc.tensor.matmul(
                    ps2[:, 0, :], lhsT=ohp, rhs=X[:, q, 48, :],
                    start=False, stop=True,
                )
                ps2sb = small.tile([2, 2, C], F32, tag="ps2sb")
                nc.scalar.copy(out=ps2sb, in_=ps2)

                # partition max per image half: transpose then 64-wide reduces
                pamx = psM.tile([128, 2, 128], F32, tag="mlp")
                for j in range(2):
                    nc.tensor.transpose(
                        pamx[:, j, :], amax[:, j * 128 : (j + 1) * 128], ident
                    )
                mxT = small.tile([128, 2, 2], F32, tag="mxT")
                nc.vector.tensor_reduce(
                    out=mxT,
                    in_=pamx.rearrange("p j (i h) -> p j i h", i=2),
                    axis=AX.X, op=OP.max,
                )
                # transpose both sum rows, folding the two t-parity halves
                # in psum: pavg[c128, j, i] = sum_a ps2sb[i, a, j-chunk]^T
                pavg = psM.tile([128, 2, 2], F32, tag="mlp")
                for j in range(2):
                    for a in range(2):
                        nc.tensor.matmul(
                            pavg[:, j, :],
                            lhsT=ps2sb[:, a, j * 128 : (j + 1) * 128],
                            rhs=ident[0:2, 0:2],
                            start=(a == 0), stop=(a == 1),
                        )
                # statsT per image [c(2x128), j, (avg, max)]
                statsTs = []
                for i in range(2):
                    statsT = small.tile([128, 2, 2], F32, tag=f"statsT{i}")
                    nc.scalar.activation(
                        out=statsT[:, :, 0:1], in_=pavg[:, :, i : i + 1],
                        func=ACT.Copy, scale=1.0 / HW,
                    )
                    nc.scalar.copy(out=statsT[:, :, 1:2], in_=mxT[:, :, i : i + 1])
                    statsTs.append(statsT)

                # ---------------- MLP per image ----------------
                caT_both = small.tile([128, 4], F32, tag="caT_both")
                for i in range(2):
                    statsT = statsTs[i]
                    ph = psM.tile([16, 2], F32, tag="mlp")
                    for j in range(2):
                        nc.tensor.matmul(
                            ph, lhsT=w1_sb[:, j, :], rhs=statsT[:, j, :],
                            start=(j == 0), stop=(j == 1),
                        )
                    h_sb = small.tile([16, 2], F32, tag="h_sb")
                    nc.scalar.activation(
                        out=h_sb, in_=ph, func=ACT.Relu, bias=b1_sb, scale=1.0
                    )
                    for j in range(2):
                        pc = psM.tile([128, 2], F32, tag="mlp")
                        nc.tensor.matmul(
                            pc, lhsT=w2_sb[:, j, :], rhs=h_sb, start=True, stop=True
                        )
                        # (avg-path + 2*b2) via ACT bias, then sigmoid with the
                        # partial sum as bias — no DVE in the chain
                        catmp = small.tile([128, 1], F32, tag="catmp")
                        nc.scalar.activation(
                            out=catmp, in_=pc[:, 0:1], func=ACT.Identity,
                            bias=b2x2[:, j : j + 1], scale=1.0,
                        )
                        nc.scalar.activation(
                            out=caT_both[:, 2 * i + j : 2 * i + j + 1],
                            in_=pc[:, 1:2], func=ACT.Sigmoid,
                            bias=catmp, scale=1.0,
                        )

                pca4 = psM.tile([4, 128], F32, tag="mlp")
                nc.tensor.transpose(pca4, caT_both, ident)
                ca4 = small.tile([4, 128], F32, tag="ca4")
                nc.scalar.copy(out=ca4, in_=pca4)
                pbca = psB.tile([P, 2, 128], F32, tag="pbca")
                nc.tensor.matmul(pbca[:, 0, :], lhsT=sel0, rhs=ca4, start=True, stop=True)
                nc.tensor.matmul(pbca[:, 1, :], lhsT=sel1, rhs=ca4, start=True, stop=True)
                bca = work.tile([P, C], F32, tag="bca")
                nc.scalar.copy(out=bca, in_=pbca.rearrange("p j m -> p (j m)"))
                bca_all.append(bca)

            # ---------------- phase B (in place, t-quarters) ----------------
            saf_all = []
            for q in range(NPAIR):
                bca = bca_all[q]
                maxc = mappool.tile([P, T], F32, tag="maxc")
                sumc = mappool.tile([P, T], F32, tag="sumc")
                for t0, t1 in ((0, 8), (8, 20), (20, 34), (34, T)):
                    nc.gpsimd.tensor_mul(
                        out=X[:, q, t0:t1, :],
                        in0=X[:, q, t0:t1, :],
                        in1=_bcast_t(bca[:, :], t1 - t0),
                    )
                    nc.vector.tensor_reduce(
                        out=maxc[:, t0:t1], in_=X[:, q, t0:t1, :],
                        axis=AX.X, op=OP.max,
                    )
                    nc.vector.tensor_reduce(
                        out=sumc[:, t0:t1], in_=X[:, q, t0:t1, :],
                        axis=AX.X, op=OP.add,
                    )

                # ---- rearrange maps: [128, 49] -> per-image [112(w,ch), 56(h)]
                mq = nc.sync if q == 0 else nc.scalar
                mdr = dpool.tile([2, P * T], F32, tag="mdr")
                mq.dma_start(
                    out=mdr[0, :].rearrange("(p t) -> p t", p=128), in_=sumc
                )
                mq.dma_start(
                    out=mdr[1, :].rearrange("(p t) -> p t", p=128), in_=maxc
                )
                sdr = dpool.tile([P * T], F32, tag="sdr")
                for i in range(2):
                    # conv input, h-major, w zero-padded by 3 on both sides
                    cinH = work.tile([112, 62], F32, tag="cinH")
                    nc.vector.memset(cinH[:, 0:3], 0.0)
                    nc.vector.memset(cinH[:, 59:62], 0.0)
                    for ch in range(2):
                        mq.dma_start(
                            out=cinH[56 * ch : 56 * (ch + 1), 3:59],
                            in_=mdr[ch, i * HW : (i + 1) * HW].rearrange(
                                "(h w) -> h w", h=56
                            ),
                        )
                    # ---- conv: 7 full-width accumulated matmuls ----
                    pconv = psB.tile([56, 56], F32, tag="pconv")
                    for dwi in range(7):
                        nc.tensor.matmul(
                            pconv,
                            lhsT=t_sb2[:, dwi, :],
                            rhs=cinH[:, dwi : dwi + 56],
                            start=(dwi == 0), stop=(dwi == 6),
                        )
                    sawh = work.tile([56, 56], F32, tag="sawh")
                    nc.scalar.activation(out=sawh, in_=pconv, func=ACT.Sigmoid)
                    mq.dma_start(
                        out=sdr[i * HW : (i + 1) * HW].rearrange("(h w) -> h w", h=56),
                        in_=sawh,
                    )
                saf = mappool.tile([P, T], F32, tag="saf")
                mq.dma_start(out=saf, in_=sdr.rearrange("(p t) -> p t", p=128))
                saf_all.append(saf)

            # ---------------- apply (in place) + DMA out ----------------
            for q in range(NPAIR):
                saf = saf_all[q]
                dst = out_flat[q * P * T : (q + 1) * P * T, :].rearrange(
                    "(p t) c -> p t c", p=128
                )
                # quarters: pair 0 on GPSIMD (DVE still on pair-1 stats);
                # pair 1 fans the tail across DVE / ACT-blocks / GPSIMD
                oq = nc.sync if q == 0 else nc.scalar
                engs = ((nc.gpsimd,) * 4 if q == 0
                        else (nc.gpsimd, nc.gpsimd, nc.vector, nc.vector))
                for (t0, t1), eng in zip(((0, 13), (13, TH), (TH, 37), (37, T)), engs):
                    if eng is None:
                        for t in range(t0, t1):
                            nc.scalar.activation(
                                out=X[:, q, t, :], in_=X[:, q, t, :],
                                func=ACT.Copy, scale=saf[:, t : t + 1],
                            )
                    else:
                        eng.tensor_mul(
                            out=X[:, q, t0:t1, :],
                            in0=X[:, q, t0:t1, :],
                            in1=_bcast_c(saf[:, t0:t1], C),
                        )
                    oq.dma_start(out=dst[:, t0:t1, :], in_=X[:, q, t0:t1, :])

    nc.finalize()
    return nc


LAST_RESULTS = None


def kernel(x, w1, b1, w2, b2, conv_w):
    global LAST_RESULTS
    nc = _CACHE.get("nc")
    if nc is None:
        nc = _build_nc()
        _CACHE["nc"] = nc

    x = np.ascontiguousarray(np.asarray(x, dtype=np.float32))
    shards = np.split(x, NCORES, axis=0)
    # host-expanded h-Toeplitz bands: tmat[56*ch + h_in, dwi, h_out] =
    #   conv_w[h_in - h_out + 3, dwi, ch] for |h_in - h_out| <= 3 else 0
    cw = np.asarray(conv_w, dtype=np.float32).reshape(7, 7, 2)
    tmat = np.zeros((2, 56, 7, 56), dtype=np.float32)
    for dhi in range(7):
        dh = dhi - 3
        for h_out in range(max(0, -dh), 56 - max(0, dh)):
            tmat[:, h_out + dh, :, h_out] = cw[dhi, :, :].T
    tmat[0] /= C  # fold the mean-map 1/C into the conv weights
    common = {
        "w1": np.ascontiguousarray(np.asarray(w1, dtype=np.float32)),
        "b1": np.ascontiguousarray(np.asarray(b1, dtype=np.float32)),
        "w2": np.ascontiguousarray(np.asarray(w2, dtype=np.float32)),
        "b2": np.ascontiguousarray(np.asarray(b2, dtype=np.float32)),
        "tmat": np.ascontiguousarray(tmat.reshape(112, 7, 56)),
    }
    in_maps = [dict(common, x=np.ascontiguousarray(s)) for s in shards]

    res = run_bass_kernel_spmd(
        nc,
        in_maps,
        core_ids=list(range(NCORES)),
        trace=bool(int(os.environ.get("CBAM_TRACE", "0"))),
    )
    LAST_RESULTS = res
    return np.concatenate([r["out"] for r in res.results], axis=0)
